# revision 16
# baseline (speedup 1.0000x reference)
"""AttentionPairBias kernel for 8 Trainium2 NeuronCores.

Sharding: data-parallel over (batch, query-row-block). Core c handles batch
b = c // 4 and query rows i in [(c % 4) * 128, (c % 4 + 1) * 128).

Design:
  - z arrives host-transposed as fp16 [c_z, i, j]; all compute in fp16 with
    fp32 PSUM accumulation.
  - LayerNorm decomposition: bias[h,i,j] = rsig(i,j) * zu'(h,i,j) + t[h]
    where u'[:,h] = ln_g*wz[:,h] - su[h]/128 (host-precomputed fp16) and
    rsig = 1/sqrt(var+eps) host-precomputed. Phase 1 is a single matmul per
    (row, col-group) with a never-changing stationary, 4-way column tiling.
  - zu' round-trips through DRAM as fp16 to flip [head,(i,j)] -> [i,j].
  - Projections keep the activation (sT / kinT chunk) stationary so one
    LDWEIGHTS serves several wide matmuls; transposed layouts (qT/kT/gT) are
    recovered with whole-tile DMA xbar transposes. Projection matmul blocks
    are emitted interleaved with the phase-1 octets so the in-order PE queue
    overlaps them with the z DMA window.
  - Softmax without max-subtraction; exp's per-partition bias carries t[h],
    accum_out produces the softmax sum, 1/sum is folded into p before the
    p-transpose, and the output accumulates directly in the transposed
    [c=(h,d), i] layout that the wo matmul needs as stationary.
"""

import sys

sys.path.insert(0, "/opt/trn_rl_repo")

from contextlib import ExitStack

import numpy as np

import concourse.bacc as bacc
import concourse.bass as bass
import concourse.mybir as mybir
import concourse.tile as tile
from concourse.bass_utils import run_bass_kernel_spmd
from concourse.masks import make_identity

F32 = mybir.dt.float32
F16 = mybir.dt.float16
AF = mybir.ActivationFunctionType
ALU = mybir.AluOpType

B, N, CS, CZ, H, D = 2, 512, 1024, 128, 16, 64
ROWS = 128          # query rows per core
NCHUNK = CS // 128  # 8 contraction chunks of 128
N_CORES = 8
EPS = 1e-5
QR = 4              # rows per (octet, col-group)

_CACHE = {}


def _build_program(mask_trivial: bool):
    nc = bacc.Bacc("TRN2", target_bir_lowering=False, debug=False,
                   num_devices=N_CORES)

    def din(name, shape):
        return nc.dram_tensor(name, shape, F32, kind="ExternalInput").ap()

    # fp16 data packed pairwise into f32-typed tensors (PJRT prefers f32).
    z16_d = din("z16", (CZ, ROWS, N // 2))
    rsig_d = din("rsig16", (ROWS, N // 2))
    sT_d = din("sT16", (128, NCHUNK, ROWS // 2))
    kinT_d = din("kinT16", (128, NCHUNK, N // 2))
    w_d = {}
    for wname in ("wq", "wk", "wv", "wg", "wo"):
        w_d[wname] = din(wname + "16", (128, NCHUNK, CS // 2))
    bq_d = din("bq8r", (1, CS // 2))
    u_d = din("u16", (CZ, 8))
    e16_d = din("e16", (16, NCHUNK, 64))
    t_d = din("trow", (128, H))
    if not mask_trivial:
        mneg_d = din("mneg16", (128, N // 2))
    out_d = nc.dram_tensor("out", (ROWS, CS), F32, kind="ExternalOutput").ap()

    with tile.TileContext(nc) as tc, ExitStack() as ctx:
        dram = ctx.enter_context(tc.tile_pool(name="dram", bufs=1, space="DRAM"))
        zu_d = dram.tile([H, ROWS, N], F16)   # zu' per head, [h, i, j]

        const = ctx.enter_context(tc.tile_pool(name="const", bufs=1))
        u_in = const.tile([CZ, 8], F32)
        nc.gpsimd.dma_start(u_in[:], u_d[:])
        t_b = const.tile([128, H], F32)
        nc.gpsimd.dma_start(t_b[:], t_d[:])
        bq8r = const.tile([1, CS // 2], F32)
        nc.gpsimd.dma_start(bq8r[:], bq_d[:])
        rsig16 = const.tile([ROWS, N // 2], F32)
        nc.sync.dma_start(rsig16[:], rsig_d[:])
        rsig = rsig16[:].bitcast(F16)

        # stationary for the z matmul: [u' (16 cols) | zeros (16)]
        u32 = const.tile([CZ, 32], F16)
        nc.vector.memset(u32[:], 0.0)
        nc.vector.tensor_copy(u32[:, 0:16], u_in[:].bitcast(F16))
        ones1 = const.tile([1, 128], F16)
        nc.vector.memset(ones1[:], 1.0)
        ident = const.tile([128, 128], F16)
        make_identity(nc, ident[:])
        e16_sb = const.tile([16, NCHUNK, 64], F32)
        nc.gpsimd.dma_start(e16_sb[:], e16_d[:])
        e16 = e16_sb[:].bitcast(F16)
        if not mask_trivial:
            mfull16 = const.tile([128, N // 2], F32)
            nc.gpsimd.dma_start(mfull16[:], mneg_d[:])
            mfull = mfull16[:].bitcast(F16)

        # big loads staged up-front on the gpsimd (SWDGE) ring; weights are
        # split in halves so projection blocks can start as soon as their
        # half has landed. wo is needed last, so it loads last.
        proj = ctx.enter_context(tc.tile_pool(name="proj", bufs=1))
        sT16 = proj.tile([128, NCHUNK, ROWS // 2], F32)
        nc.gpsimd.dma_start(sT16[:], sT_d[:])
        kinT16 = proj.tile([128, NCHUNK, N // 2], F32)
        nc.gpsimd.dma_start(kinT16[:], kinT_d[:])
        w_sbs = {}
        for wname in ("wq", "wg", "wk", "wv", "wo"):
            t_w = proj.tile([128, NCHUNK, CS // 2], F32, name=f"w_{wname}")
            for hf in range(2):
                nc.gpsimd.dma_start(t_w[:, :, 256 * hf:256 * hf + 256],
                                    w_d[wname][:, :, 256 * hf:256 * hf + 256])
            w_sbs[wname] = t_w

        def w16(wname):
            return w_sbs[wname][:].bitcast(F16)

        sT = sT16[:].bitcast(F16)       # [128, 8, 128]
        kinT = kinT16[:].bitcast(F16)   # [128, 8, 512]

        # ------------- phase 2 tiles (filled during phase 1) -------------
        att = ctx.enter_context(tc.tile_pool(name="att", bufs=1))
        qT16 = att.tile([128, NCHUNK, ROWS], F16)   # (q+bq)/8 transposed [d, i]
        kT16 = att.tile([128, NCHUNK, N], F16)      # k transposed [d, j]
        gT16 = att.tile([128, NCHUNK, ROWS], F16)   # sigmoid(s@wg).T [c, i]
        v16 = att.tile([128, 4, CS], F16)           # [j in chunk, jc, h*64+d]
        q_sb = att.tile([128, CS], F16)             # q/8 + bq/8, [i, d]
        g_sb = att.tile([128, CS], F16)             # sigmoid(s@wg), [i, c]
        k_sb = att.tile([128, 4, CS], F16)          # k, [j, jc, d]

        prps = ctx.enter_context(tc.tile_pool(name="prps", bufs=2, space="PSUM"))

        def qg_block(nh):
            # q and g, half nh: stationary = sT chunk, moving = wq/wg halves
            psq = prps.tile([128, 512], F32, tag="p2", name=f"psq{nh}")
            psg = prps.tile([128, 512], F32, tag="p2", name=f"psg{nh}")
            for cc in range(NCHUNK):
                first, last = cc == 0, cc == NCHUNK - 1
                nc.tensor.matmul(psq[:], sT[:, cc, :],
                                 w16("wq")[:, cc, 512 * nh:512 * nh + 512],
                                 start=first, stop=False)
                nc.tensor.matmul(psg[:], sT[:, cc, :],
                                 w16("wg")[:, cc, 512 * nh:512 * nh + 512],
                                 start=first, stop=last)
            nc.tensor.matmul(psq[:], ones1[:],
                             bq8r[:].bitcast(F16)[:, 512 * nh:512 * nh + 512],
                             start=False, stop=True)
            nc.vector.tensor_copy(q_sb[:, 512 * nh:512 * nh + 512], psq[:])
            nc.scalar.activation(g_sb[:, 512 * nh:512 * nh + 512], psg[:],
                                 AF.Sigmoid)
            if nh == 1:
                nc.scalar.dma_start_transpose(qT16[:], q_sb[:])
                nc.sync.dma_start_transpose(gT16[:], g_sb[:])

        def kv_block(jc):
            # k and v for one j-chunk: stationary = kinT chunk, 4 wide movers
            pk = [prps.tile([128, 512], F32, tag="p2", name=f"pk{jc}_{i}")
                  for i in range(2)]
            pv = [prps.tile([128, 512], F32, tag="p2", name=f"pv{jc}_{i}")
                  for i in range(2)]
            for cc in range(NCHUNK):
                first, last = cc == 0, cc == NCHUNK - 1
                for nh in range(2):
                    nc.tensor.matmul(
                        pk[nh][:], kinT[:, cc, 128 * jc:128 * jc + 128],
                        w16("wk")[:, cc, 512 * nh:512 * nh + 512],
                        start=first, stop=last)
                    nc.tensor.matmul(
                        pv[nh][:], kinT[:, cc, 128 * jc:128 * jc + 128],
                        w16("wv")[:, cc, 512 * nh:512 * nh + 512],
                        start=first, stop=last)
            for nh in range(2):
                nc.vector.tensor_copy(
                    k_sb[:, jc, 512 * nh:512 * nh + 512], pk[nh][:])
                nc.vector.tensor_copy(
                    v16[:, jc, 512 * nh:512 * nh + 512], pv[nh][:])
            ring = nc.scalar if jc % 2 == 0 else nc.sync
            ring.dma_start_transpose(kT16[:, :, 128 * jc:128 * jc + 128],
                                     k_sb[:, jc, :])

        # ------- phase 1: z -> zu' (DRAM, fp16), projections woven in -------
        p2blocks = {2: lambda: qg_block(0), 3: lambda: qg_block(1),
                    4: lambda: kv_block(0), 5: lambda: kv_block(1),
                    6: lambda: kv_block(2), 7: lambda: kv_block(3)}
        with ExitStack() as zctx:
            ztp = zctx.enter_context(tc.tile_pool(name="ztp", bufs=10))
            zup = zctx.enter_context(tc.tile_pool(name="zup", bufs=3))
            zps = zctx.enter_context(tc.tile_pool(name="zps", bufs=4, space="PSUM"))

            for o in range(32 // QR):
                zins = []
                for g in range(4):
                    r0 = 32 * g + QR * o
                    zin = ztp.tile([CZ, QR, N // 2], F32, tag="zin")
                    nc.sync.dma_start(zin[:], z16_d[:, r0:r0 + QR, :])
                    zins.append(zin)
                zu_sb = zup.tile([128, QR, N], F16, tag="zu")
                for kk in range(QR):
                    ps = zps.tile([128, N], F32, tag="pzu")
                    for g in range(4):
                        mv = zins[g][:, kk, :].bitcast(F16)  # [CZ, N]
                        nc.tensor.matmul(ps[32 * g:32 * g + 32, :], u32[:], mv,
                                         start=True, stop=True,
                                         tile_position=(0, 32 * g))
                    nc.vector.tensor_copy(zu_sb[:, kk, :], ps[:])
                for g in range(4):
                    r0 = 32 * g + QR * o
                    nc.scalar.dma_start(zu_d[0:16, r0:r0 + QR, :],
                                        zu_sb[32 * g:32 * g + 16, :, :])
                if o in p2blocks:
                    p2blocks[o]()

        # ---------------- phase 3: attention ----------------
        ap3 = ctx.enter_context(tc.tile_pool(name="ap3", bufs=1))
        zhp = ctx.enter_context(tc.tile_pool(name="zhp", bufs=3))
        sp3 = ctx.enter_context(tc.tile_pool(name="sp3", bufs=4))
        php = ctx.enter_context(tc.tile_pool(name="php", bufs=3))
        ptp = ctx.enter_context(tc.tile_pool(name="ptp", bufs=3))
        spsum = ctx.enter_context(tc.tile_pool(name="spsum", bufs=2, space="PSUM"))
        opsum = ctx.enter_context(tc.tile_pool(name="opsum", bufs=1, space="PSUM"))
        rcps = ctx.enter_context(tc.tile_pool(name="rcps", bufs=2, space="PSUM"))

        # o accumulated transposed: [c = (h, d), i], chunked by cc = h // 2
        oT_ps = opsum.tile([128, NCHUNK, ROWS], F32)
        sums = ap3.tile([128, H], F32)

        for m in range(H // 2):
            # two heads (2m, 2m+1) per iteration share one zu load and one
            # p-transpose so the fixed DMA costs amortize.
            zu_h2 = zhp.tile([128, 2, N], F16, tag="zh")
            nc.sync.dma_start(zu_h2[:],
                              zu_d[2 * m:2 * m + 2, :, :]
                              .rearrange("o i j -> i o j"))
            p2 = php.tile([128, 2, N], F16, tag="ph")
            for hh in range(2):
                h = 2 * m + hh
                p0 = 64 * hh
                sc_ps = spsum.tile([128, N], F32, tag="sc")
                nc.tensor.matmul(sc_ps[:],
                                 qT16[p0:p0 + 64, m, :],
                                 kT16[p0:p0 + 64, m, :],
                                 start=True, stop=True)
                s2 = sp3.tile([128, N], F16, tag="s2")
                nc.vector.tensor_tensor(s2[:], zu_h2[:, hh, :], rsig, ALU.mult)
                if not mask_trivial:
                    nc.vector.tensor_tensor(s2[:], s2[:], mfull, ALU.add)
                s3 = sp3.tile([128, N], F16, tag="s3")
                nc.vector.tensor_tensor(s3[:], s2[:], sc_ps[:], ALU.add)
                nc.scalar.activation(p2[:, hh, :], s3[:], AF.Exp,
                                     bias=t_b[:, h:h + 1],
                                     accum_out=sums[:, h:h + 1])
            ptT = ptp.tile([128, 8, ROWS], F16, tag="pt")
            nc.sync.dma_start_transpose(ptT[:], p2[:])
            for jc in range(4):
                for hh in range(2):
                    h = 2 * m + hh
                    p0 = 64 * hh
                    nc.tensor.matmul(oT_ps[p0:p0 + 64, m, :],
                                     v16[:, jc, D * h:D * h + D],
                                     ptT[:, 4 * hh + jc, :],
                                     start=(jc == 0), stop=(jc == 3),
                                     tile_position=(0, p0))

        # softmax denominators: transpose 1/sums into [h, i] and broadcast
        # each head's row over its 64 d-partitions with a rank-2 matmul.
        sums16 = ap3.tile([128, H], F16)
        nc.vector.tensor_copy(sums16[:], sums[:])
        sT_ps = rcps.tile([16, 128], F16, tag="rc", name="sT_ps")
        nc.tensor.transpose(sT_ps[:], sums16[:], ident[:])
        rcT16 = ap3.tile([16, 128], F16)
        with nc.allow_low_precision(reason="softmax denom reciprocal in fp16"):
            nc.vector.reciprocal(rcT16[:], sT_ps[:])

        goT = ap3.tile([128, NCHUNK, ROWS], F16)
        for cc in range(NCHUNK):
            rcb = rcps.tile([128, 128], F32, tag="rc", name=f"rcb{cc}")
            nc.tensor.matmul(rcb[:], e16[:, cc, :], rcT16[:],
                             start=True, stop=True)
            rcb_sb = ap3.tile([128, 128], F16, name=f"rcbs{cc}")
            nc.scalar.copy(rcb_sb[:], rcb[:])
            tmp = ap3.tile([128, 128], F16, name=f"gtmp{cc}")
            nc.vector.tensor_tensor(tmp[:], oT_ps[:, cc, :], rcb_sb[:], ALU.mult)
            eng = nc.vector if cc % 2 == 0 else nc.gpsimd
            eng.tensor_tensor(goT[:, cc, :], tmp[:], gT16[:, cc, :], ALU.mult)

        out_sb = ap3.tile([128, CS], F32)
        for nh in range(2):
            ps = spsum.tile([128, 512], F32, tag="sc")
            for cc in range(NCHUNK):
                nc.tensor.matmul(ps[:], goT[:, cc, :],
                                 w16("wo")[:, cc, 512 * nh:512 * nh + 512],
                                 start=(cc == 0), stop=(cc == NCHUNK - 1))
            nc.vector.tensor_copy(out_sb[:, 512 * nh:512 * nh + 512], ps[:])
        nc.sync.dma_start(out_d[:], out_sb[:])

    nc.compile()
    return nc


def _e16_const():
    e = np.zeros((16, NCHUNK, 128), dtype=np.float32)
    for cc in range(NCHUNK):
        e[2 * cc, cc, 0:64] = 1.0
        e[2 * cc + 1, cc, 64:128] = 1.0
    return e


def _pack16(a):
    a16 = np.ascontiguousarray(np.asarray(a, dtype=np.float16))
    return a16.view(np.float32)


def _prepare(s, z, mask, k_in, wq, bq, wk, wv, wg, ln_g, ln_b, wz, wo,
             multiplicity=1, **_ignored):
    s = np.asarray(s, dtype=np.float32)
    z = np.asarray(z, dtype=np.float32)
    mask = np.asarray(mask, dtype=np.float32)
    k_in = np.asarray(k_in, dtype=np.float32)
    assert int(multiplicity) == 1, "only multiplicity == 1 is supported"
    mask_trivial = bool(np.all(mask == 1.0))

    def wchunk16(w):
        w = np.asarray(w, dtype=np.float32).reshape(NCHUNK, 128, CS) \
            .transpose(1, 0, 2)
        return _pack16(w)

    u = np.asarray(ln_g, np.float32)[:, None] * np.asarray(wz, np.float32)
    su = u.sum(axis=0)
    up = u - su[None, :] / CZ
    trow = np.ascontiguousarray(np.broadcast_to(
        (np.asarray(ln_b, np.float32) @ np.asarray(wz, np.float32))
        .reshape(1, H), (128, H)).astype(np.float32))

    shared = {
        "wq16": wchunk16(np.asarray(wq, np.float32) * 0.125),
        "wk16": wchunk16(wk), "wv16": wchunk16(wv),
        "wg16": wchunk16(wg), "wo16": wchunk16(wo),
        "bq8r": _pack16((np.asarray(bq, np.float32) * 0.125).reshape(1, CS)),
        "u16": _pack16(up),
        "e16": _pack16(_e16_const()),
        "trow": trow,
    }
    in_maps = []
    for core in range(N_CORES):
        b, ib = core // 4, core % 4
        i0 = ib * ROWS
        m = dict(shared)
        m["sT16"] = _pack16(
            s[b, i0:i0 + ROWS, :].T.reshape(NCHUNK, 128, ROWS)
            .transpose(1, 0, 2))
        m["kinT16"] = _pack16(
            k_in[b].T.reshape(NCHUNK, 128, N).transpose(1, 0, 2))
        zs = z[b, i0:i0 + ROWS]                       # [i, j, c]
        m["z16"] = _pack16(zs.transpose(2, 0, 1))     # [c, i, j] fp16
        var = zs.var(axis=2)                          # [i, j] over c
        m["rsig16"] = _pack16(1.0 / np.sqrt(var + EPS))
        if not mask_trivial:
            m["mneg16"] = _pack16(np.broadcast_to(
                ((1.0 - mask[b]) * -30000.0).reshape(1, N), (128, N)))
        in_maps.append(m)
    return mask_trivial, in_maps


def _run(in_maps, mask_trivial, **kwargs):
    if mask_trivial not in _CACHE:
        _CACHE[mask_trivial] = _build_program(mask_trivial)
    nc = _CACHE[mask_trivial]
    res = run_bass_kernel_spmd(nc, in_maps, core_ids=list(range(N_CORES)),
                               **kwargs)
    out = np.empty((B, N, CS), dtype=np.float32)
    for core in range(N_CORES):
        b, ib = core // 4, core % 4
        out[b, ib * ROWS:(ib + 1) * ROWS, :] = res.results[core]["out"]
    return out, res


def kernel(**inputs):
    mask_trivial, in_maps = _prepare(**inputs)
    out, _ = _run(in_maps, mask_trivial)
    return out


def run_profiled(inputs, tmpdir=None):
    mask_trivial, in_maps = _prepare(**inputs)
    out, res = _run(in_maps, mask_trivial, trace=True, tmpdir=tmpdir)
    return out, res


# revision 19
# speedup vs baseline: 1.0592x; 1.0592x over previous
"""AttentionPairBias kernel for 8 Trainium2 NeuronCores.

Sharding: data-parallel over (batch, query-row-block). Core c handles batch
b = c // 4 and query rows i in [(c % 4) * 128, (c % 4 + 1) * 128).

Design:
  - z arrives host-transposed as fp16 [c_z, i, j]; all compute in fp16 with
    fp32 PSUM accumulation.
  - LayerNorm decomposition: bias[h,i,j] = rsig(i,j) * zu'(h,i,j) + t[h]
    where u'[:,h] = ln_g*wz[:,h] - su[h]/128 (host-precomputed fp16) and
    rsig = 1/sqrt(var+eps) host-precomputed. Phase 1 is a single matmul per
    (row, col-group) with a never-changing stationary, 4-way column tiling.
  - zu' round-trips through DRAM as fp16 to flip [head,(i,j)] -> [i,j].
  - Projections keep the activation (sT / kinT chunk) stationary so one
    LDWEIGHTS serves several wide matmuls; transposed layouts (qT/kT/gT) are
    recovered with whole-tile DMA xbar transposes. Projection matmul blocks
    are emitted interleaved with the phase-1 octets so the in-order PE queue
    overlaps them with the z DMA window.
  - Softmax without max-subtraction; exp's per-partition bias carries t[h],
    accum_out produces the softmax sum, 1/sum is folded into p before the
    p-transpose, and the output accumulates directly in the transposed
    [c=(h,d), i] layout that the wo matmul needs as stationary.
"""

import sys

sys.path.insert(0, "/opt/trn_rl_repo")

from contextlib import ExitStack

import numpy as np

import concourse.bacc as bacc
import concourse.bass as bass
import concourse.mybir as mybir
import concourse.tile as tile
from concourse.bass_utils import run_bass_kernel_spmd
from concourse.masks import make_identity

F32 = mybir.dt.float32
F16 = mybir.dt.float16
AF = mybir.ActivationFunctionType
ALU = mybir.AluOpType

B, N, CS, CZ, H, D = 2, 512, 1024, 128, 16, 64
ROWS = 128          # query rows per core
NCHUNK = CS // 128  # 8 contraction chunks of 128
N_CORES = 8
EPS = 1e-5
QR = 4              # rows per (octet, col-group)

_CACHE = {}


def _build_program(mask_trivial: bool):
    nc = bacc.Bacc("TRN2", target_bir_lowering=False, debug=False,
                   num_devices=N_CORES)

    def din(name, shape):
        return nc.dram_tensor(name, shape, F32, kind="ExternalInput").ap()

    # fp16 data packed pairwise into f32-typed tensors (PJRT prefers f32).
    z16_d = din("z16", (CZ, ROWS, N // 2))
    rsig_d = din("rsig16", (ROWS, N // 2))
    sT_d = din("sT16", (128, NCHUNK, ROWS // 2))
    kinT_d = din("kinT16", (128, NCHUNK, N // 2))
    w_d = {}
    for wname in ("wq", "wk", "wv", "wg", "wo"):
        w_d[wname] = din(wname + "16", (128, NCHUNK, CS // 2))
    bq_d = din("bq8r", (1, CS // 2))
    u_d = din("u16", (CZ, 8))
    e16_d = din("e16", (16, NCHUNK, 64))
    t_d = din("trow", (128, H))
    if not mask_trivial:
        mneg_d = din("mneg16", (128, N // 2))
    out_d = nc.dram_tensor("out", (ROWS, CS), F32, kind="ExternalOutput").ap()

    with tile.TileContext(nc) as tc, ExitStack() as ctx:
        dram = ctx.enter_context(tc.tile_pool(name="dram", bufs=1, space="DRAM"))
        zu_d = dram.tile([H, ROWS, N], F16)   # zu' per head, [h, i, j]

        const = ctx.enter_context(tc.tile_pool(name="const", bufs=1))
        u_in = const.tile([CZ, 8], F32)
        nc.gpsimd.dma_start(u_in[:], u_d[:])
        t_b = const.tile([128, H], F32)
        nc.gpsimd.dma_start(t_b[:], t_d[:])
        bq8r = const.tile([1, CS // 2], F32)
        nc.gpsimd.dma_start(bq8r[:], bq_d[:])
        rsig16 = const.tile([ROWS, N // 2], F32)
        nc.sync.dma_start(rsig16[:], rsig_d[:])
        rsig = rsig16[:].bitcast(F16)

        # stationary for the z matmul: [u' (16 cols) | zeros (16)]
        u32 = const.tile([CZ, 32], F16)
        nc.vector.memset(u32[:], 0.0)
        nc.vector.tensor_copy(u32[:, 0:16], u_in[:].bitcast(F16))
        ones1 = const.tile([1, 128], F16)
        nc.vector.memset(ones1[:], 1.0)
        ident = const.tile([128, 128], F16)
        make_identity(nc, ident[:])
        e16_sb = const.tile([16, NCHUNK, 64], F32)
        nc.gpsimd.dma_start(e16_sb[:], e16_d[:])
        e16 = e16_sb[:].bitcast(F16)
        if not mask_trivial:
            mfull16 = const.tile([128, N // 2], F32)
            nc.gpsimd.dma_start(mfull16[:], mneg_d[:])
            mfull = mfull16[:].bitcast(F16)

        # big loads staged up-front on the gpsimd (SWDGE) ring; weights are
        # split in halves so projection blocks can start as soon as their
        # half has landed. wo is needed last, so it loads last.
        proj = ctx.enter_context(tc.tile_pool(name="proj", bufs=1))
        sT16 = proj.tile([128, NCHUNK, ROWS // 2], F32)
        nc.gpsimd.dma_start(sT16[:], sT_d[:])
        kinT16 = proj.tile([128, NCHUNK, N // 2], F32)
        nc.gpsimd.dma_start(kinT16[:], kinT_d[:])
        w_sbs = {}
        for wname in ("wq", "wg", "wk", "wv", "wo"):
            t_w = proj.tile([128, NCHUNK, CS // 2], F32, name=f"w_{wname}")
            for hf in range(2):
                nc.gpsimd.dma_start(t_w[:, :, 256 * hf:256 * hf + 256],
                                    w_d[wname][:, :, 256 * hf:256 * hf + 256])
            w_sbs[wname] = t_w

        def w16(wname):
            return w_sbs[wname][:].bitcast(F16)

        sT = sT16[:].bitcast(F16)       # [128, 8, 128]
        kinT = kinT16[:].bitcast(F16)   # [128, 8, 512]

        # ------------- phase 2 tiles (filled during phase 1) -------------
        att = ctx.enter_context(tc.tile_pool(name="att", bufs=1))
        qT16 = att.tile([128, NCHUNK, ROWS], F16)   # (q+bq)/8 transposed [d, i]
        kT16 = att.tile([128, NCHUNK, N], F16)      # k transposed [d, j]
        gT16 = att.tile([128, NCHUNK, ROWS], F16)   # sigmoid(s@wg).T [c, i]
        v16 = att.tile([128, 4, CS], F16)           # [j in chunk, jc, h*64+d]
        q_sb = att.tile([128, CS], F16)             # q/8 + bq/8, [i, d]
        g_sb = att.tile([128, CS], F16)             # sigmoid(s@wg), [i, c]
        k_sb = att.tile([128, 4, CS], F16)          # k, [j, jc, d]

        prps = None  # created inside the phase-1 pool stack below

        def q_piece():
            psq = [prps.tile([128, 512], F32, tag="p2", name=f"psq{i}")
                   for i in range(2)]
            for nh in range(2):
                for cc in range(NCHUNK):
                    nc.tensor.matmul(psq[nh][:], sT[:, cc, :],
                                     w16("wq")[:, cc, 512 * nh:512 * nh + 512],
                                     start=(cc == 0), stop=False)
                nc.tensor.matmul(psq[nh][:], ones1[:],
                                 bq8r[:].bitcast(F16)[:, 512 * nh:512 * nh + 512],
                                 start=False, stop=True)
                nc.vector.tensor_copy(q_sb[:, 512 * nh:512 * nh + 512],
                                      psq[nh][:])
            nc.scalar.dma_start_transpose(qT16[:], q_sb[:])

        def g_piece():
            psg = [prps.tile([128, 512], F32, tag="p2", name=f"psg{i}")
                   for i in range(2)]
            for nh in range(2):
                for cc in range(NCHUNK):
                    nc.tensor.matmul(psg[nh][:], sT[:, cc, :],
                                     w16("wg")[:, cc, 512 * nh:512 * nh + 512],
                                     start=(cc == 0), stop=(cc == NCHUNK - 1))
                nc.scalar.activation(g_sb[:, 512 * nh:512 * nh + 512],
                                     psg[nh][:], AF.Sigmoid)
            nc.sync.dma_start_transpose(gT16[:], g_sb[:])

        def k_piece(jcs):
            for jc in jcs:
                pk = [prps.tile([128, 512], F32, tag="p2", name=f"pk{jc}_{i}")
                      for i in range(2)]
                for cc in range(NCHUNK):
                    first, last = cc == 0, cc == NCHUNK - 1
                    for nh in range(2):
                        nc.tensor.matmul(
                            pk[nh][:], kinT[:, cc, 128 * jc:128 * jc + 128],
                            w16("wk")[:, cc, 512 * nh:512 * nh + 512],
                            start=first, stop=last)
                for nh in range(2):
                    nc.vector.tensor_copy(
                        k_sb[:, jc, 512 * nh:512 * nh + 512], pk[nh][:])
                ring = nc.scalar if jc % 2 == 0 else nc.sync
                ring.dma_start_transpose(kT16[:, :, 128 * jc:128 * jc + 128],
                                         k_sb[:, jc, :])

        def v_piece(jcs):
            for jc in jcs:
                pv = [prps.tile([128, 512], F32, tag="p2", name=f"pv{jc}_{i}")
                      for i in range(2)]
                for cc in range(NCHUNK):
                    first, last = cc == 0, cc == NCHUNK - 1
                    for nh in range(2):
                        nc.tensor.matmul(
                            pv[nh][:], kinT[:, cc, 128 * jc:128 * jc + 128],
                            w16("wv")[:, cc, 512 * nh:512 * nh + 512],
                            start=first, stop=last)
                for nh in range(2):
                    nc.vector.tensor_copy(
                        v16[:, jc, 512 * nh:512 * nh + 512], pv[nh][:])

        # ------- phase 1: z -> zu' (DRAM, fp16), projections woven in -------
        p2blocks = {2: q_piece, 3: g_piece,
                    4: lambda: k_piece((0, 1)), 5: lambda: k_piece((2, 3)),
                    6: lambda: v_piece((0, 1)), 7: lambda: v_piece((2, 3))}
        with ExitStack() as zctx:
            ztp = zctx.enter_context(tc.tile_pool(name="ztp", bufs=10))
            zup = zctx.enter_context(tc.tile_pool(name="zup", bufs=3))
            zps = zctx.enter_context(tc.tile_pool(name="zps", bufs=3, space="PSUM"))
            prps = zctx.enter_context(tc.tile_pool(name="prps", bufs=4,
                                                   space="PSUM"))

            for o in range(32 // QR):
                zins = []
                for g in range(4):
                    r0 = 32 * g + QR * o
                    zin = ztp.tile([CZ, QR, N // 2], F32, tag="zin")
                    nc.sync.dma_start(zin[:], z16_d[:, r0:r0 + QR, :])
                    zins.append(zin)
                zu_sb = zup.tile([128, QR, N], F16, tag="zu")
                for kk in range(QR):
                    ps = zps.tile([128, N], F32, tag="pzu")
                    for g in range(4):
                        mv = zins[g][:, kk, :].bitcast(F16)  # [CZ, N]
                        nc.tensor.matmul(ps[32 * g:32 * g + 32, :], u32[:], mv,
                                         start=True, stop=True,
                                         tile_position=(0, 32 * g))
                    nc.vector.tensor_copy(zu_sb[:, kk, :], ps[:])
                for g in range(4):
                    r0 = 32 * g + QR * o
                    nc.scalar.dma_start(zu_d[0:16, r0:r0 + QR, :],
                                        zu_sb[32 * g:32 * g + 16, :, :])
                if o in p2blocks:
                    p2blocks[o]()

        # ---------------- phase 3: attention ----------------
        ap3 = ctx.enter_context(tc.tile_pool(name="ap3", bufs=1))
        zhp = ctx.enter_context(tc.tile_pool(name="zhp", bufs=4))
        sp3 = ctx.enter_context(tc.tile_pool(name="sp3", bufs=4))
        php = ctx.enter_context(tc.tile_pool(name="php", bufs=3))
        ptp = ctx.enter_context(tc.tile_pool(name="ptp", bufs=3))
        spsum = ctx.enter_context(tc.tile_pool(name="spsum", bufs=2, space="PSUM"))
        opsum = ctx.enter_context(tc.tile_pool(name="opsum", bufs=1, space="PSUM"))
        rcps = ctx.enter_context(tc.tile_pool(name="rcps", bufs=2, space="PSUM"))

        # o accumulated transposed: [c = (h, d), i], chunked by cc = h // 2
        oT_ps = opsum.tile([128, NCHUNK, ROWS], F32)
        sums = ap3.tile([128, H], F32)

        for m in range(H // 2):
            # two heads (2m, 2m+1) per iteration share one zu load and one
            # p-transpose so the fixed DMA costs amortize.
            zu_h2 = zhp.tile([128, 2, N], F16, tag="zh")
            nc.sync.dma_start(zu_h2[:],
                              zu_d[2 * m:2 * m + 2, :, :]
                              .rearrange("o i j -> i o j"))
            p2 = php.tile([128, 2, N], F16, tag="ph")
            for hh in range(2):
                h = 2 * m + hh
                p0 = 64 * hh
                sc_ps = spsum.tile([128, N], F32, tag="sc")
                nc.tensor.matmul(sc_ps[:],
                                 qT16[p0:p0 + 64, m, :],
                                 kT16[p0:p0 + 64, m, :],
                                 start=True, stop=True)
                s2 = sp3.tile([128, N], F16, tag="s2")
                nc.vector.tensor_tensor(s2[:], zu_h2[:, hh, :], rsig, ALU.mult)
                if not mask_trivial:
                    nc.vector.tensor_tensor(s2[:], s2[:], mfull, ALU.add)
                s3 = sp3.tile([128, N], F16, tag="s3")
                nc.vector.tensor_tensor(s3[:], s2[:], sc_ps[:], ALU.add)
                nc.scalar.activation(p2[:, hh, :], s3[:], AF.Exp,
                                     bias=t_b[:, h:h + 1],
                                     accum_out=sums[:, h:h + 1])
            ptT = ptp.tile([128, 8, ROWS], F16, tag="pt")
            nc.sync.dma_start_transpose(ptT[:], p2[:])
            for jc in range(4):
                for hh in range(2):
                    h = 2 * m + hh
                    p0 = 64 * hh
                    nc.tensor.matmul(oT_ps[p0:p0 + 64, m, :],
                                     v16[:, jc, D * h:D * h + D],
                                     ptT[:, 4 * hh + jc, :],
                                     start=(jc == 0), stop=(jc == 3),
                                     tile_position=(0, p0))

        # softmax denominators: transpose 1/sums into [h, i] and broadcast
        # each head's row over its 64 d-partitions with a rank-2 matmul.
        sums16 = ap3.tile([128, H], F16)
        nc.vector.tensor_copy(sums16[:], sums[:])
        sT_ps = rcps.tile([16, 128], F16, tag="rc", name="sT_ps")
        nc.tensor.transpose(sT_ps[:], sums16[:], ident[:])
        rcT16 = ap3.tile([16, 128], F16)
        with nc.allow_low_precision(reason="softmax denom reciprocal in fp16"):
            nc.vector.reciprocal(rcT16[:], sT_ps[:])

        goT = ap3.tile([128, NCHUNK, ROWS], F16)
        for cc in range(NCHUNK):
            rcb = rcps.tile([128, 128], F32, tag="rc", name=f"rcb{cc}")
            nc.tensor.matmul(rcb[:], e16[:, cc, :], rcT16[:],
                             start=True, stop=True)
            rcb_sb = ap3.tile([128, 128], F16, name=f"rcbs{cc}")
            nc.scalar.copy(rcb_sb[:], rcb[:])
            tmp = ap3.tile([128, 128], F16, name=f"gtmp{cc}")
            nc.vector.tensor_tensor(tmp[:], oT_ps[:, cc, :], rcb_sb[:], ALU.mult)
            eng = nc.vector if cc % 2 == 0 else nc.gpsimd
            eng.tensor_tensor(goT[:, cc, :], tmp[:], gT16[:, cc, :], ALU.mult)

        out_sb = ap3.tile([128, CS], F32)
        for nh in range(2):
            ps = spsum.tile([128, 512], F32, tag="sc")
            for cc in range(NCHUNK):
                nc.tensor.matmul(ps[:], goT[:, cc, :],
                                 w16("wo")[:, cc, 512 * nh:512 * nh + 512],
                                 start=(cc == 0), stop=(cc == NCHUNK - 1))
            nc.vector.tensor_copy(out_sb[:, 512 * nh:512 * nh + 512], ps[:])
        nc.sync.dma_start(out_d[:], out_sb[:])

    nc.compile()
    return nc


def _e16_const():
    e = np.zeros((16, NCHUNK, 128), dtype=np.float32)
    for cc in range(NCHUNK):
        e[2 * cc, cc, 0:64] = 1.0
        e[2 * cc + 1, cc, 64:128] = 1.0
    return e


def _pack16(a):
    a16 = np.ascontiguousarray(np.asarray(a, dtype=np.float16))
    return a16.view(np.float32)


def _prepare(s, z, mask, k_in, wq, bq, wk, wv, wg, ln_g, ln_b, wz, wo,
             multiplicity=1, **_ignored):
    s = np.asarray(s, dtype=np.float32)
    z = np.asarray(z, dtype=np.float32)
    mask = np.asarray(mask, dtype=np.float32)
    k_in = np.asarray(k_in, dtype=np.float32)
    assert int(multiplicity) == 1, "only multiplicity == 1 is supported"
    mask_trivial = bool(np.all(mask == 1.0))

    def wchunk16(w):
        w = np.asarray(w, dtype=np.float32).reshape(NCHUNK, 128, CS) \
            .transpose(1, 0, 2)
        return _pack16(w)

    u = np.asarray(ln_g, np.float32)[:, None] * np.asarray(wz, np.float32)
    su = u.sum(axis=0)
    up = u - su[None, :] / CZ
    trow = np.ascontiguousarray(np.broadcast_to(
        (np.asarray(ln_b, np.float32) @ np.asarray(wz, np.float32))
        .reshape(1, H), (128, H)).astype(np.float32))

    shared = {
        "wq16": wchunk16(np.asarray(wq, np.float32) * 0.125),
        "wk16": wchunk16(wk), "wv16": wchunk16(wv),
        "wg16": wchunk16(wg), "wo16": wchunk16(wo),
        "bq8r": _pack16((np.asarray(bq, np.float32) * 0.125).reshape(1, CS)),
        "u16": _pack16(up),
        "e16": _pack16(_e16_const()),
        "trow": trow,
    }
    in_maps = []
    for core in range(N_CORES):
        b, ib = core // 4, core % 4
        i0 = ib * ROWS
        m = dict(shared)
        m["sT16"] = _pack16(
            s[b, i0:i0 + ROWS, :].T.reshape(NCHUNK, 128, ROWS)
            .transpose(1, 0, 2))
        m["kinT16"] = _pack16(
            k_in[b].T.reshape(NCHUNK, 128, N).transpose(1, 0, 2))
        zs = z[b, i0:i0 + ROWS]                       # [i, j, c]
        m["z16"] = _pack16(zs.transpose(2, 0, 1))     # [c, i, j] fp16
        var = zs.var(axis=2)                          # [i, j] over c
        m["rsig16"] = _pack16(1.0 / np.sqrt(var + EPS))
        if not mask_trivial:
            m["mneg16"] = _pack16(np.broadcast_to(
                ((1.0 - mask[b]) * -30000.0).reshape(1, N), (128, N)))
        in_maps.append(m)
    return mask_trivial, in_maps


def _run(in_maps, mask_trivial, **kwargs):
    if mask_trivial not in _CACHE:
        _CACHE[mask_trivial] = _build_program(mask_trivial)
    nc = _CACHE[mask_trivial]
    res = run_bass_kernel_spmd(nc, in_maps, core_ids=list(range(N_CORES)),
                               **kwargs)
    out = np.empty((B, N, CS), dtype=np.float32)
    for core in range(N_CORES):
        b, ib = core // 4, core % 4
        out[b, ib * ROWS:(ib + 1) * ROWS, :] = res.results[core]["out"]
    return out, res


def kernel(**inputs):
    mask_trivial, in_maps = _prepare(**inputs)
    out, _ = _run(in_maps, mask_trivial)
    return out


def run_profiled(inputs, tmpdir=None):
    mask_trivial, in_maps = _prepare(**inputs)
    out, res = _run(in_maps, mask_trivial, trace=True, tmpdir=tmpdir)
    return out, res
